# revision 26
# baseline (speedup 1.0000x reference)
"""Balanced dice loss (histogram binning) on 8 Trainium2 NeuronCores.

Math: with t ∈ {0,1} and p = sigmoid(x), the loss needs four global sums:
    S_t   = Σ t            (the bincount)
    S_pt  = Σ p·t
    S_pp  = Σ p²
    S_ppt = Σ p²·t
Then with c1 = S_t, c0 = N − c1, w0 = 1/(c0+s)², w1 = 1/(c1+s)²:
    intersection = w1·S_pt
    denominator  = w0·(S_pp − S_ppt) + w1·(S_ppt + c1)
    dice = 1 − (2·I + s)/(D + s)

Device kernel (data-parallel over 8 cores, batch-sharded), per [128,F] tile:
    ACT : p = sigmoid(x) (bf16); tb = copy(t) int32→bf16 with row-accum
          → S_t (one pass converts dtype AND takes the bincount);
          square(p) on the first FH columns with row-accum → S_pp part 1
    DVE : u = p·tb, w = u·p (= p²·t) in bf16 2× perf mode; sq = p·p on
          the remaining columns + f32 row-reduce → S_pp part 2
    PE  : ones[128,128] @ 512-col chunks of u and w → two PSUM column-sum
          accumulation chains (S_pt, S_ppt), each alternating two banks
          to pipeline the PSUM RMW
The split is sized so every engine stays under the ~82µs HBM stream
(32 MB/core at ~410 GB/s measured) in EVERY PE clock-gate (HAM) state:
128 matmuls fit the stream even fully cold at 1.2 GHz, which removes the
throttle-dependent run-to-run spread seen with a third matmul chain.
Partials are DMA'd out; host reduces in float64 and finishes the scalar
math.
"""

import numpy as np

import concourse.bacc as bacc
import concourse.mybir as mybir
from concourse.bass_utils import run_bass_kernel_spmd
from concourse.tile import TileContext

N_CORES = 8
P = 128
TOTAL = 32 * 1024 * 1024  # elements in the full problem
PER_CORE = TOTAL // N_CORES  # 4,194,304
FREE = PER_CORE // P  # 32,768 f32 per partition
F = 2048  # tile free-dim
NT = FREE // F  # 16 tiles per core
MMN = 512  # matmul moving free-dim (one PSUM bank; ISA max)
NCH = F // MMN  # matmul chunks per tile
FH = F // 4  # S_pp split point: [:FH] on ACT, [FH:] on DVE
SMOOTH = 1e-05

_nc_cache = None


def _build_bass():
    nc = bacc.Bacc(None, target_bir_lowering=False)
    x = nc.dram_tensor("input", [P, FREE], mybir.dt.float32, kind="ExternalInput")
    t = nc.dram_tensor("target", [P, FREE], mybir.dt.int32, kind="ExternalInput")
    o_sums = nc.dram_tensor(
        "o_sums", [1, 4 * MMN], mybir.dt.float32, kind="ExternalOutput"
    )
    o_st = nc.dram_tensor("o_st", [P, NT], mybir.dt.float32, kind="ExternalOutput")
    o_pp = nc.dram_tensor("o_pp", [P, 2 * NT], mybir.dt.float32, kind="ExternalOutput")

    with TileContext(nc) as tc:
        with (
            tc.tile_pool(name="work", bufs=2) as pool,
            tc.tile_pool(name="stats", bufs=1) as spool,
            tc.tile_pool(name="ps", bufs=1, space="PSUM") as psum,
        ):
            s_t = spool.tile([P, NT], mybir.dt.float32)
            s_pp = spool.tile([P, 2 * NT], mybir.dt.float32)
            ones = spool.tile([P, P], mybir.dt.bfloat16, tag="ones")
            bias0 = spool.tile([P, 1], mybir.dt.float32, tag="bias0")
            junk = spool.tile([P, FH], mybir.dt.bfloat16, tag="junk")
            ps_pt_a = psum.tile([P, MMN], mybir.dt.float32, tag="ps_pt_a")
            ps_pt_b = psum.tile([P, MMN], mybir.dt.float32, tag="ps_pt_b")
            ps_ppt_a = psum.tile([P, MMN], mybir.dt.float32, tag="ps_ppt_a")
            ps_ppt_b = psum.tile([P, MMN], mybir.dt.float32, tag="ps_ppt_b")

            # pre-allocated rotating buffers (manual multi-buffering keeps
            # the pool-bookkeeping semaphore count — and with it the
            # end-of-kernel teardown drain — small)
            NXB, NPB, NWB = 6, 3, 2

            def mktiles(base, n, dt, cols=F):
                return [
                    pool.tile(
                        [P, cols], dt, tag=f"{base}{k}", name=f"{base}{k}", bufs=1
                    )
                    for k in range(n)
                ]

            xts = mktiles("xt", NXB, mybir.dt.float32)
            tts = mktiles("tt", NXB, mybir.dt.int32)
            pbufs = mktiles("p", NPB, mybir.dt.bfloat16)
            tbufs = mktiles("tb", NPB, mybir.dt.bfloat16)
            ubufs = mktiles("u", NWB, mybir.dt.bfloat16)
            wbufs = mktiles("w", NWB, mybir.dt.bfloat16)
            sqbufs = mktiles("sq", NWB, mybir.dt.bfloat16, cols=F - FH)

            # emit the first tile's loads before the ones-memset so the
            # sync queue reaches them as early as possible
            for i in range(NT):
                xt, tt = xts[i % NXB], tts[i % NXB]
                nc.sync.dma_start(xt[:], x[:, i * F : (i + 1) * F])
                nc.sync.dma_start(tt[:], t[:, i * F : (i + 1) * F])
                if i == 0:
                    nc.any.memset(ones, 1.0)
                    nc.any.memset(bias0, 0.0)

            for i in range(NT):
                xt, tt = xts[i % NXB], tts[i % NXB]
                p_, tb = pbufs[i % NPB], tbufs[i % NPB]
                u, w, sq = ubufs[i % NWB], wbufs[i % NWB], sqbufs[i % NWB]

                # p = sigmoid(x); tb = float(t) with S_t row-accum   [ACT]
                nc.scalar.activation(
                    p_[:],
                    xt[:],
                    mybir.ActivationFunctionType.Sigmoid,
                    bias=bias0[:, 0:1],
                )
                nc.scalar.activation(
                    tb[:],
                    tt[:],
                    mybir.ActivationFunctionType.Copy,
                    accum_out=s_t[:, i : i + 1],
                )
                # u = p·t, w = u·p = p²t (bf16, 2x mode); the last tile
                # runs in 512-col sub-slices to shorten the drain    [DVE]
                if i < NT - 1:
                    dve_slices = [slice(0, F)]
                else:
                    dve_slices = [slice(c * MMN, (c + 1) * MMN) for c in range(NCH)]
                for sl in dve_slices:
                    nc.vector.tensor_tensor(
                        out=u[:, sl], in0=p_[:, sl], in1=tb[:, sl],
                        op=mybir.AluOpType.mult,
                    )
                    nc.vector.tensor_tensor(
                        out=w[:, sl], in0=u[:, sl], in1=p_[:, sl],
                        op=mybir.AluOpType.mult,
                    )
                    # column-sum accumulation chains; each chain
                    # alternates two PSUM banks to pipeline the RMW  [PE]
                    j0 = sl.start // MMN
                    nch = (sl.stop - sl.start) // MMN
                    for s_, banks in (
                        (u, (ps_pt_a, ps_pt_b)),
                        (w, (ps_ppt_a, ps_ppt_b)),
                    ):
                        for jj in range(nch):
                            j = j0 + jj
                            nc.tensor.matmul(
                                banks[j % 2][:],
                                ones[:],
                                s_[:, j * MMN : (j + 1) * MMN],
                                start=(i == 0 and j < 2),
                                stop=(i == NT - 1 and j >= NCH - 2),
                            )
                # S_pp: [:FH] via ACT square row-accum …             [ACT]
                nc.scalar.activation(
                    junk[:],
                    p_[:, :FH],
                    mybir.ActivationFunctionType.Square,
                    bias=bias0[:, 0:1],
                    accum_out=s_pp[:, 2 * i : 2 * i + 1],
                )
                # … and [FH:] via DVE square + f32 row-reduce        [DVE]
                nc.vector.tensor_tensor(
                    out=sq[:], in0=p_[:, FH:], in1=p_[:, FH:], op=mybir.AluOpType.mult
                )
                nc.vector.tensor_reduce(
                    s_pp[:, 2 * i + 1 : 2 * i + 2],
                    sq[:],
                    axis=mybir.AxisListType.X,
                    op=mybir.AluOpType.add,
                )

            fin = spool.tile([1, 4 * MMN], mybir.dt.float32, tag="fin")
            for k, ps in enumerate((ps_pt_a, ps_pt_b, ps_ppt_a, ps_ppt_b)):
                dst = fin[:, k * MMN : (k + 1) * MMN]
                if k % 2 == 0:
                    nc.vector.tensor_copy(dst, ps[0:1, :])
                else:
                    nc.scalar.copy(dst, ps[0:1, :])
            nc.sync.dma_start(o_sums[:], fin[:])
            nc.sync.dma_start(o_st[:], s_t[:])
            nc.sync.dma_start(o_pp[:], s_pp[:])
    nc.finalize()
    return nc


def _get_nc():
    global _nc_cache
    if _nc_cache is None:
        _nc_cache = _build_bass()
    return _nc_cache


def kernel(input, target, _trace=False):
    x = np.ascontiguousarray(np.asarray(input, dtype=np.float32)).reshape(
        N_CORES, P, FREE
    )
    t = np.ascontiguousarray(np.asarray(target, dtype=np.int32)).reshape(
        N_CORES, P, FREE
    )
    in_maps = [{"input": x[i], "target": t[i]} for i in range(N_CORES)]

    nc = _get_nc()
    res = run_bass_kernel_spmd(
        nc, in_maps, core_ids=list(range(N_CORES)), trace=_trace
    )
    kernel.last_results = res

    s_pt = s_ppt = s_pp = s_t = 0.0
    for r in res.results:
        sums = r["o_sums"].astype(np.float64)
        s_pt += float(sums[0, 0 : 2 * MMN].sum())
        s_ppt += float(sums[0, 2 * MMN :].sum())
        s_pp += float(r["o_pp"].astype(np.float64).sum())
        s_t += float(r["o_st"].astype(np.float64).sum())

    c1 = float(s_t)
    c0 = float(TOTAL - s_t)
    w0 = 1.0 / (c0 + SMOOTH) ** 2
    w1 = 1.0 / (c1 + SMOOTH) ** 2
    intersection = w1 * s_pt
    denominator = w0 * (s_pp - s_ppt) + w1 * (s_ppt + c1)
    dice = 1.0 - (2.0 * intersection + SMOOTH) / (denominator + SMOOTH)
    return np.asarray(dice, dtype=np.float32)


# revision 27
# speedup vs baseline: 1.6667x; 1.6667x over previous
"""Balanced dice loss (histogram binning) on 8 Trainium2 NeuronCores.

Math: with t ∈ {0,1} and p = sigmoid(x), the loss needs four global sums:
    S_t   = Σ t            (the bincount)
    S_pt  = Σ p·t
    S_pp  = Σ p²
    S_ppt = Σ p²·t
Then with c1 = S_t, c0 = N − c1, w0 = 1/(c0+s)², w1 = 1/(c1+s)²:
    intersection = w1·S_pt
    denominator  = w0·(S_pp − S_ppt) + w1·(S_ppt + c1)
    dice = 1 − (2·I + s)/(D + s)

Device kernel (data-parallel over 8 cores, batch-sharded), per [128,F] tile:
    ACT : p = sigmoid(x) (bf16); tb = copy(t) int32→bf16 with row-accum
          → S_t (one pass converts dtype AND takes the bincount);
          square(p) on the first FH columns with row-accum → S_pp part 1
    DVE : u = p·tb, w = u·p (= p²·t) in bf16 2× perf mode; sq = p·p on
          the remaining columns + f32 row-reduce → S_pp part 2
    PE  : ones[128,128] @ 512-col chunks of u and w → two PSUM column-sum
          accumulation chains (S_pt, S_ppt), each alternating two banks
          to pipeline the PSUM RMW
The split is sized so every engine stays under the ~82µs HBM stream
(32 MB/core at ~410 GB/s measured) in EVERY PE clock-gate (HAM) state:
128 matmuls fit the stream even fully cold at 1.2 GHz, which removes the
throttle-dependent run-to-run spread seen with a third matmul chain.
Partials are DMA'd out; host reduces in float64 and finishes the scalar
math.
"""

import numpy as np

import concourse.bacc as bacc
import concourse.mybir as mybir
from concourse.bass_utils import run_bass_kernel_spmd
from concourse.tile import TileContext

N_CORES = 8
P = 128
TOTAL = 32 * 1024 * 1024  # elements in the full problem
PER_CORE = TOTAL // N_CORES  # 4,194,304
FREE = PER_CORE // P  # 32,768 f32 per partition
F = 2048  # tile free-dim
NT = FREE // F  # 16 tiles per core
MMN = 512  # matmul moving free-dim (one PSUM bank; ISA max)
NCH = F // MMN  # matmul chunks per tile
FH = F // 4  # S_pp split point: [:FH] on ACT, [FH:] on DVE
SMOOTH = 1e-05

_nc_cache = None


def _build_bass():
    nc = bacc.Bacc(None, target_bir_lowering=False)
    x = nc.dram_tensor("input", [P, FREE], mybir.dt.float32, kind="ExternalInput")
    t = nc.dram_tensor("target", [P, FREE], mybir.dt.int32, kind="ExternalInput")
    o_sums = nc.dram_tensor(
        "o_sums", [1, 4 * MMN], mybir.dt.float32, kind="ExternalOutput"
    )
    o_st = nc.dram_tensor("o_st", [P, NT], mybir.dt.float32, kind="ExternalOutput")
    o_pp = nc.dram_tensor("o_pp", [P, 2 * NT], mybir.dt.float32, kind="ExternalOutput")

    with TileContext(nc) as tc:
        with (
            tc.tile_pool(name="work", bufs=2) as pool,
            tc.tile_pool(name="stats", bufs=1) as spool,
            tc.tile_pool(name="ps", bufs=1, space="PSUM") as psum,
        ):
            s_t = spool.tile([P, NT], mybir.dt.float32)
            s_pp = spool.tile([P, 2 * NT], mybir.dt.float32)
            ones = spool.tile([P, P], mybir.dt.bfloat16, tag="ones")
            bias0 = spool.tile([P, 1], mybir.dt.float32, tag="bias0")
            junk = spool.tile([P, FH], mybir.dt.bfloat16, tag="junk")
            ps_pt_a = psum.tile([P, MMN], mybir.dt.float32, tag="ps_pt_a")
            ps_pt_b = psum.tile([P, MMN], mybir.dt.float32, tag="ps_pt_b")
            ps_ppt_a = psum.tile([P, MMN], mybir.dt.float32, tag="ps_ppt_a")
            ps_ppt_b = psum.tile([P, MMN], mybir.dt.float32, tag="ps_ppt_b")

            # emit the first tile's loads before the ones-memset so the
            # sync queue reaches them as early as possible
            xts, tts = [], []
            for i in range(NT):
                xt = pool.tile([P, F], mybir.dt.float32, tag="xt", bufs=6)
                tt = pool.tile([P, F], mybir.dt.int32, tag="tt", bufs=6)
                nc.sync.dma_start(xt[:], x[:, i * F : (i + 1) * F])
                nc.sync.dma_start(tt[:], t[:, i * F : (i + 1) * F])
                xts.append(xt)
                tts.append(tt)
                if i == 0:
                    nc.any.memset(ones, 1.0)
                    nc.any.memset(bias0, 0.0)

            for i in range(NT):
                xt, tt = xts[i], tts[i]
                p_ = pool.tile([P, F], mybir.dt.bfloat16, tag="p", bufs=3)
                tb = pool.tile([P, F], mybir.dt.bfloat16, tag="tb", bufs=3)
                u = pool.tile([P, F], mybir.dt.bfloat16, tag="u")
                w = pool.tile([P, F], mybir.dt.bfloat16, tag="w")
                sq = pool.tile([P, F - FH], mybir.dt.bfloat16, tag="sq")

                # p = sigmoid(x); tb = float(t) with S_t row-accum   [ACT]
                nc.scalar.activation(
                    p_[:],
                    xt[:],
                    mybir.ActivationFunctionType.Sigmoid,
                    bias=bias0[:, 0:1],
                )
                nc.scalar.activation(
                    tb[:],
                    tt[:],
                    mybir.ActivationFunctionType.Copy,
                    accum_out=s_t[:, i : i + 1],
                )
                # u = p·t, w = u·p = p²t (bf16, 2x mode); the last tile
                # runs in 512-col sub-slices to shorten the drain    [DVE]
                if i < NT - 1:
                    dve_slices = [slice(0, F)]
                else:
                    dve_slices = [slice(c * MMN, (c + 1) * MMN) for c in range(NCH)]
                for sl in dve_slices:
                    nc.vector.tensor_tensor(
                        out=u[:, sl], in0=p_[:, sl], in1=tb[:, sl],
                        op=mybir.AluOpType.mult,
                    )
                    nc.vector.tensor_tensor(
                        out=w[:, sl], in0=u[:, sl], in1=p_[:, sl],
                        op=mybir.AluOpType.mult,
                    )
                    # column-sum accumulation chains; each chain
                    # alternates two PSUM banks to pipeline the RMW  [PE]
                    j0 = sl.start // MMN
                    nch = (sl.stop - sl.start) // MMN
                    for s_, banks in (
                        (u, (ps_pt_a, ps_pt_b)),
                        (w, (ps_ppt_a, ps_ppt_b)),
                    ):
                        for jj in range(nch):
                            j = j0 + jj
                            nc.tensor.matmul(
                                banks[j % 2][:],
                                ones[:],
                                s_[:, j * MMN : (j + 1) * MMN],
                                start=(i == 0 and j < 2),
                                stop=(i == NT - 1 and j >= NCH - 2),
                            )
                # S_pp: [:FH] via ACT square row-accum …             [ACT]
                nc.scalar.activation(
                    junk[:],
                    p_[:, :FH],
                    mybir.ActivationFunctionType.Square,
                    bias=bias0[:, 0:1],
                    accum_out=s_pp[:, 2 * i : 2 * i + 1],
                )
                # … and [FH:] via DVE square + f32 row-reduce        [DVE]
                nc.vector.tensor_tensor(
                    out=sq[:], in0=p_[:, FH:], in1=p_[:, FH:], op=mybir.AluOpType.mult
                )
                nc.vector.tensor_reduce(
                    s_pp[:, 2 * i + 1 : 2 * i + 2],
                    sq[:],
                    axis=mybir.AxisListType.X,
                    op=mybir.AluOpType.add,
                )

            fin = spool.tile([1, 4 * MMN], mybir.dt.float32, tag="fin")
            for k, ps in enumerate((ps_pt_a, ps_pt_b, ps_ppt_a, ps_ppt_b)):
                dst = fin[:, k * MMN : (k + 1) * MMN]
                if k % 2 == 0:
                    nc.vector.tensor_copy(dst, ps[0:1, :])
                else:
                    nc.scalar.copy(dst, ps[0:1, :])
            nc.sync.dma_start(o_sums[:], fin[:])
            nc.sync.dma_start(o_st[:], s_t[:])
            nc.sync.dma_start(o_pp[:], s_pp[:])
    nc.finalize()
    return nc


def _get_nc():
    global _nc_cache
    if _nc_cache is None:
        _nc_cache = _build_bass()
    return _nc_cache


def kernel(input, target, _trace=False):
    x = np.ascontiguousarray(np.asarray(input, dtype=np.float32)).reshape(
        N_CORES, P, FREE
    )
    t = np.ascontiguousarray(np.asarray(target, dtype=np.int32)).reshape(
        N_CORES, P, FREE
    )
    in_maps = [{"input": x[i], "target": t[i]} for i in range(N_CORES)]

    nc = _get_nc()
    res = run_bass_kernel_spmd(
        nc, in_maps, core_ids=list(range(N_CORES)), trace=_trace
    )
    kernel.last_results = res

    s_pt = s_ppt = s_pp = s_t = 0.0
    for r in res.results:
        sums = r["o_sums"].astype(np.float64)
        s_pt += float(sums[0, 0 : 2 * MMN].sum())
        s_ppt += float(sums[0, 2 * MMN :].sum())
        s_pp += float(r["o_pp"].astype(np.float64).sum())
        s_t += float(r["o_st"].astype(np.float64).sum())

    c1 = float(s_t)
    c0 = float(TOTAL - s_t)
    w0 = 1.0 / (c0 + SMOOTH) ** 2
    w1 = 1.0 / (c1 + SMOOTH) ** 2
    intersection = w1 * s_pt
    denominator = w0 * (s_pp - s_ppt) + w1 * (s_ppt + c1)
    dice = 1.0 - (2.0 * intersection + SMOOTH) / (denominator + SMOOTH)
    return np.asarray(dice, dtype=np.float32)
